# revision 18
# baseline (speedup 1.0000x reference)
"""Cross-attention (B=4, N=2048, C=768, H=12, HD=64) on 8 TRN2 NeuronCores.

Sharding: core = (batch, head_group) with 4 batches x 2 groups of 6 heads.
Each core computes its group's Q/K/V projections, per-head-dim LayerNorm,
attention, and a partial output projection; the host sums the two group
partials per batch and adds the bias.

Key optimizations over the f32r baseline:
 - All matmul operands are bf16 (PSUM accumulation stays f32).  The PE
   processes one moving row per cycle either way, but bf16 halves SBUF
   footprint, weight-load time, and DMA traffic.
 - Query-token compaction: the reference masks ~50% of QUERY rows, and a
   masked row's output is exactly the uniform average of v (its score row
   is all zeros after the mask is folded into the LN scale).  The host
   gathers unmasked q tokens, the kernel runs attention on only NQ ~ 1280
   columns, and any PAD column (rs=0) computes precisely the uniform-
   attention output -- so the host reads column cnt_b as the shared
   output for all masked tokens of batch b, then scatters.
 - Attention scores are computed transposed (S^T[k_tok, q_tok]); softmax
   exp needs no row-max (LN bounds |S|); the denominator comes free from
   a ones-column appended to v.
"""

import numpy as np
import ml_dtypes

import concourse.bass as bass
import concourse.mybir as mybir
from concourse import tile
from concourse import bass_utils
from concourse.tile_scheduler import N_PROCS
from concourse.vector_clock import ScopedClock, VectorClock

F32 = mybir.dt.float32
BF16 = mybir.dt.bfloat16
AF = mybir.ActivationFunctionType
OP = mybir.AluOpType
NPBF16 = ml_dtypes.bfloat16

B, N, C, H, HD = 4, 2048, 768, 12, 64
G = 2                 # head groups (tensor parallel)
HPG = H // G          # 6 heads per group
CL = HPG * HD         # 384 local channels
P = 128
NT = CL // P          # 3 output tiles per group
CT = C // P           # 6 contraction tiles
TT = N // P           # 16 k-token tiles
EPS = 1e-5
SCALE = HD ** -0.5
NCORES = 8

_nop_ctr = [0]


class _FixedTileContext(tile.TileContext):
    """Workaround for a walrus build that allows at most ONE sync-wait per
    instruction: split multi-wait instructions into single-wait NoOps on the
    same engine, and emit the kernel-tail drain's waits as a nop chain."""

    def _split_multiwait(self, insts):
        out = []
        for inst in insts:
            si = getattr(inst, "sync_info", None)
            waits = list(si.on_wait) if si is not None and si.on_wait else []
            if len(waits) > 1:
                eng = inst.engine
                for w in waits[:-1]:
                    _nop_ctr[0] += 1
                    nop = mybir.InstNoOp(
                        name=f"I-waitsplit-{_nop_ctr[0]}", ins=[], outs=[]
                    )
                    nop.engine = eng
                    nop.sync_info = mybir.SyncInfo(on_wait=[w], on_update=[])
                    self.nc.register_instruction(nop)
                    out.append(nop)
                inst.sync_info = mybir.SyncInfo(
                    on_wait=[waits[-1]], on_update=list(si.on_update)
                )
            out.append(inst)
        return out

    def _lower_ordered_insts(self, ordered):
        ordered = {bb: self._split_multiwait(ins) for bb, ins in ordered.items()}
        super()._lower_ordered_insts(ordered)

    def _drain_and_barrier(self, tick_clock, wait_clock):
        gc = tick_clock.global_clock
        vals = [gc[p] for p in range(N_PROCS)]
        for p in [q for q, v in enumerate(vals) if v > 0]:
            partial = VectorClock(
                [vals[q] if q == p else 0 for q in range(N_PROCS)]
            )
            nop = self.nc.sync.nop(nofuse=True, hint="tail_drain_wait")
            wait_clock.add_sem_waits(nop.ins, ScopedClock({None: partial}))
        self.nc.sync.drain()
        self.nc.all_engine_barrier()
        assert self.sems is not None
        popped = self.nc._tile_sem_poison_stack.pop()
        assert popped is self._sem_poison
        self.nc.clear_and_free_semaphores(list(self.sems.allocated().values()))
        self.nc.all_engine_barrier()


def _mm(nc, out, lhsT, rhs, start, stop):
    nc.tensor.matmul(
        out, lhsT, rhs, start=start, stop=stop, skip_group_check=True
    )


def _chunks(total):
    """Split token range into chunks of 512 with a trailing 256 if needed."""
    out = []
    off = 0
    while off < total:
        w = 512 if total - off >= 512 else total - off
        out.append((off, w))
        off += w
    return out


def _body(tc, aps, nq):
    nc = tc.nc
    qxT, kvxT, wq, wk, wv, wp, msk, colsel, bcast, ind, vones, outT = aps

    cpool = tc.alloc_tile_pool(name="consts", bufs=1)
    bpool = tc.alloc_tile_pool(name="big", bufs=1)
    w_pool = tc.alloc_tile_pool(name="wts", bufs=1)

    # weights + first x tiles first in the DMA queue, consts after
    wq_sb = w_pool.tile([P, CT, CL], BF16, name="wq", tag="wq")
    nc.sync.dma_start(wq_sb[:], wq.rearrange("(ct p) m -> p ct m", p=P))
    wp_sb = bpool.tile([P, NT, C], BF16, name="wp", tag="wp")

    q_sb = [bpool.tile([P, nq], BF16, name=f"q{t}", tag=f"q{t}") for t in range(NT)]
    k_sb = [bpool.tile([P, N], BF16, name=f"k{t}", tag=f"k{t}") for t in range(NT)]
    v_sb = bpool.tile([P, TT, HPG, HD + 1], BF16, name="v", tag="v")
    den_all = bpool.tile([65, HPG * 512], F32, name="den", tag="den")

    q_chunks = _chunks(nq)
    k_chunks = _chunks(N)

    # ---------------- phase 1: projections + layernorm ----------------
    ps_t = tc.alloc_tile_pool(name="ps1", bufs=8, space="PSUM")
    xq_pool = tc.alloc_tile_pool(name="xq", bufs=3)
    xkv_pool = tc.alloc_tile_pool(name="xkv", bufs=7)
    sq_pool = tc.alloc_tile_pool(name="sq", bufs=3)
    st_pool = tc.alloc_tile_pool(name="st", bufs=2)
    rs_pool = tc.alloc_tile_pool(name="rs", bufs=3)

    def ln_chunk_a(xT, w_sb, dst, masked, co, cw):
        """Projection + LN stats; returns context for the apply step."""
        cs = slice(co, co + cw)
        pp = [ps_t.tile([P, cw], F32, name="pt", tag="pt") for _ in range(NT)]
        xts = []
        pool = xq_pool if masked else xkv_pool
        xtag = "xq" if masked else "xkv"
        for ct in range(CT):
            xt = pool.tile([P, cw], BF16, name=xtag, tag=xtag)
            nc.sync.dma_start(xt[:], xT[ct * P:(ct + 1) * P, cs])
            xts.append(xt)
            for t in range(NT):
                _mm(nc, pp[t][:], w_sb[:, ct, t * P:(t + 1) * P],
                    xt[:], ct == 0, ct == CT - 1)
        sqs = []
        for t in range(NT):
            nc.vector.tensor_copy(dst[t][:, cs], pp[t][:])
            sq_t = sq_pool.tile([P, cw], BF16, name="sq", tag="sq")
            nc.scalar.activation(sq_t[:], pp[t][:], AF.Square)
            sqs.append(sq_t)
        mu_ps = ps_t.tile([HPG, cw], F32, name="pt", tag="pt")
        for t in range(NT):
            _mm(nc, mu_ps[:], colsel_sb[:, t, :], dst[t][:, cs],
                t == 0, t == NT - 1)
        ms_ps = ps_t.tile([HPG, cw], F32, name="pt", tag="pt")
        for t in range(NT):
            _mm(nc, ms_ps[:], colsel_sb[:, t, :], sqs[t][:],
                t == 0, t == NT - 1)
        st = st_pool.tile([HPG, 2 * cw], F32, name="st", tag="st")
        work = st[:, 0:cw]
        mu_sb = st[:, cw:2 * cw]
        rsm = rs_pool.tile([HPG, 2, cw], BF16, name="rsm", tag="rsm")
        rs = rsm[:, 0, :]
        murs = rsm[:, 1, :]
        nc.vector.tensor_copy(mu_sb, mu_ps[:])
        # var = E[x^2] - mu^2
        nc.vector.scalar_tensor_tensor(
            work, mu_sb, 1.0, mu_sb, OP.mult, OP.mult)
        nc.vector.tensor_tensor(work, ms_ps[:], work, OP.subtract)
        # rs = (var + eps)^-0.5 = exp(-0.5 * ln(var + eps))
        nc.scalar.activation(work, work, AF.Ln, bias=eps_sb[:])
        if masked:
            nc.scalar.activation(work, work, AF.Exp, scale=-0.5)
            # fold attn scale + query mask into rs
            nc.vector.tensor_tensor(rs, work, msk_sb[:, cs], OP.mult)
        else:
            nc.scalar.activation(rs, work, AF.Exp, scale=-0.5)
        # murs = -mu * rs
        nc.vector.scalar_tensor_tensor(
            murs, mu_sb, -1.0, rs, OP.mult, OP.mult)
        if not masked:
            # v projection reuses this chunk's kv x-tiles
            for tl in range(cw // P):
                tt = co // P + tl
                vp = ps_t.tile([P, CL], F32, name="pt", tag="pt")
                for ct in range(CT):
                    _mm(nc, vp[:], xts[ct][:, tl * P:(tl + 1) * P],
                        wv_sb[:, ct, :], ct == 0, ct == CT - 1)
                nc.vector.tensor_copy(
                    v_sb[:, tt, :, 0:HD],
                    vp[:].rearrange("p (h d) -> p h d", h=HPG))
        return (dst, rs, murs, co, cw)

    def ln_chunk_b(ctx):
        """Broadcast rs/murs across head partitions and normalize dst."""
        dst, rs, murs, co, cw = ctx
        cs = slice(co, co + cw)
        for t in range(NT):
            rrep = ps_t.tile([P, cw], F32, name="pt", tag="pt")
            _mm(nc, rrep[:], bcast_sb[:, t, :], rs, True, True)
            mrep = ps_t.tile([P, cw], F32, name="pt", tag="pt")
            _mm(nc, mrep[:], bcast_sb[:, t, :], murs, True, True)
            nc.vector.tensor_tensor(
                dst[t][:, cs], dst[t][:, cs], rrep[:], OP.mult)
            nc.vector.tensor_tensor(
                dst[t][:, cs], dst[t][:, cs], mrep[:], OP.add)

    if True:
        # schedule: interleave q/k chunks, software-pipelined by one stage
        sched = []
        for c in range(len(k_chunks)):
            if c < len(q_chunks):
                sched.append((qxT, None, q_sb, True, q_chunks[c]))
            sched.append((kvxT, None, k_sb, False, k_chunks[c]))

        colsel_sb = cpool.tile([P, NT, HPG], BF16, name="colsel", tag="colsel")
        nc.sync.dma_start(colsel_sb[:], colsel[:])
        bcast_sb = cpool.tile([HPG, NT, P], BF16, name="bcast", tag="bcast")
        nc.sync.dma_start(bcast_sb[:], bcast[:])
        msk_sb = cpool.tile([HPG, nq], F32, name="msk", tag="msk")
        nc.sync.dma_start(msk_sb[:], msk[:])
        eps_sb = cpool.tile([HPG, 1], F32, name="eps", tag="eps")
        nc.vector.memset(eps_sb[:], EPS)
        wk_sb = w_pool.tile([P, CT, CL], BF16, name="wk", tag="wk")
        wv_sb = w_pool.tile([P, CT, CL], BF16, name="wv", tag="wv")

        prev = None
        for i, (xT, _, dst, masked, (co, cw)) in enumerate(sched):
            w_sb = wq_sb if masked else wk_sb
            cur = ln_chunk_a(xT, w_sb, dst, masked, co, cw)
            if i == 0:
                # big weight loads ride the Activation-engine DGE queue so
                # they don't serialize behind x-tile streaming on SP
                nc.scalar.dma_start(
                    wk_sb[:], wk.rearrange("(ct p) m -> p ct m", p=P))
                nc.scalar.dma_start(
                    wv_sb[:], wv.rearrange("(ct p) m -> p ct m", p=P))
                nc.scalar.dma_start(v_sb[:, :, :, HD], vones[:])
                nc.scalar.dma_start(
                    wp_sb[:], wp.rearrange("(t p) m -> p t m", p=P))
            if prev is not None:
                ln_chunk_b(prev)
            prev = cur
        ln_chunk_b(prev)

    for pool in (rs_pool, st_pool, sq_pool, xkv_pool, xq_pool, w_pool, ps_t):
        pool.release()

    # ---------------- phase 2: attention + output projection ----------
    ps_s = tc.alloc_tile_pool(name="ps_s", bufs=2, space="PSUM")
    ps_o = tc.alloc_tile_pool(name="ps_o", bufs=2, space="PSUM")
    ps_t = tc.alloc_tile_pool(name="ps2", bufs=2, space="PSUM")
    e_pool = tc.alloc_tile_pool(name="e", bufs=4)
    o_pool = tc.alloc_tile_pool(name="o", bufs=2)
    den_pool = tc.alloc_tile_pool(name="dn", bufs=2)
    rcp_pool = tc.alloc_tile_pool(name="rcp", bufs=2)
    out_pool = tc.alloc_tile_pool(name="ot", bufs=3)
    if True:
        ind_sb = cpool.tile([HPG, NT, P], BF16, name="ind", tag="ind")
        nc.sync.dma_start(ind_sb[:], ind[:])

        def finish(ctx):
            """Normalize o_t by the softmax denominators and project out."""
            o_t, db, co, cw = ctx
            qs = slice(co, co + cw)
            den6 = rcp_pool.tile([HPG, cw], F32, name="den6", tag="den6")
            nc.sync.dma_start(den6[:], den_all[db:db + 1, 0:HPG * cw])
            rcp6 = rcp_pool.tile([HPG, cw], BF16, name="rcp6", tag="rcp6")
            with nc.allow_low_precision(reason="bf16 softmax denom recip"):
                nc.vector.reciprocal(rcp6[:], den6[:])
            for t in range(NT):
                rrep = ps_t.tile([P, cw], F32, name="pt", tag="pt")
                _mm(nc, rrep[:], ind_sb[:, t, :], rcp6[:], True, True)
                nc.vector.tensor_tensor(
                    o_t[t][:], o_t[t][:], rrep[:], OP.mult)
            for m in range(C // P):
                pp = ps_t.tile([P, cw], F32, name="pt", tag="pt")
                for t in range(NT):
                    _mm(nc, pp[:], wp_sb[:, t, m * P:(m + 1) * P],
                        o_t[t][:], t == 0, t == NT - 1)
                ot = out_pool.tile([P, cw], F32, name="ot", tag="ot")
                nc.vector.tensor_copy(ot[:], pp[:])
                nc.sync.dma_start(outT[m * P:(m + 1) * P, qs], ot[:])

        pending = None
        for qc, (co, cw) in enumerate(q_chunks):
            qs = slice(co, co + cw)
            kt_grp = 1024 // cw          # k-tiles per [128, 1024] exp group
            o_t = [o_pool.tile([P, cw], BF16, name=f"o{t}", tag=f"o{t}")
                   for t in range(NT)]
            db = 32 * (qc % 3)
            for hp in range(HPG // 2):
                t = hp
                po = [ps_o.tile([HD + 1, cw], F32, name="po", tag="po")
                      for _ in range(2)]
                for kg in range(TT // kt_grp):
                    sps, es = [], []
                    for hh in range(2):
                        off = hh * HD
                        sp = ps_s.tile([P, kt_grp * cw], F32,
                                       name="sp", tag="sp")
                        for j in range(kt_grp):
                            kt = kg * kt_grp + j
                            _mm(nc, sp[:, j * cw:(j + 1) * cw],
                                k_sb[t][off:off + HD, kt * P:(kt + 1) * P],
                                q_sb[t][off:off + HD, qs],
                                True, True)
                        sps.append(sp)
                    for hh in range(2):
                        e = e_pool.tile([P, kt_grp * cw], BF16,
                                        name="e", tag="e")
                        nc.scalar.activation(e[:], sps[hh][:], AF.Exp)
                        es.append(e)
                    for hh in range(2):
                        h = 2 * hp + hh
                        for j in range(kt_grp):
                            kt = kg * kt_grp + j
                            _mm(nc, po[hh][:], v_sb[:, kt, h, :],
                                es[hh][:, j * cw:(j + 1) * cw],
                                kt == 0, kt == TT - 1)
                for hh in range(2):
                    h = 2 * hp + hh
                    off = hh * HD
                    nc.vector.tensor_copy(
                        den_all[db:db + 1, h * cw:(h + 1) * cw],
                        po[hh][HD:HD + 1, :])
                    nc.vector.tensor_copy(
                        o_t[t][off:off + HD, :], po[hh][0:HD, :])
                if hp == 0 and pending is not None:
                    finish(pending)
                    pending = None
            pending = (o_t, db, co, cw)
        finish(pending)

    for pool in (out_pool, rcp_pool, den_pool, o_pool, e_pool,
                 ps_t, ps_o, ps_s, bpool, cpool):
        pool.release()


def build_bass(nq):
    nc = bass.Bass(trn_type="TRN2", debug=False, num_devices=NCORES)
    qxT = nc.dram_tensor("qxT", [C, nq], BF16, kind="ExternalInput").ap()
    kvxT = nc.dram_tensor("kvxT", [C, N], BF16, kind="ExternalInput").ap()
    wq = nc.dram_tensor("wq", [C, CL], BF16, kind="ExternalInput").ap()
    wk = nc.dram_tensor("wk", [C, CL], BF16, kind="ExternalInput").ap()
    wv = nc.dram_tensor("wv", [C, CL], BF16, kind="ExternalInput").ap()
    wp = nc.dram_tensor("wp", [CL, C], BF16, kind="ExternalInput").ap()
    msk = nc.dram_tensor("msk", [HPG, nq], F32, kind="ExternalInput").ap()
    colsel = nc.dram_tensor("colsel", [P, NT, HPG], BF16,
                            kind="ExternalInput").ap()
    bcast = nc.dram_tensor("bcast", [HPG, NT, P], BF16,
                           kind="ExternalInput").ap()
    ind = nc.dram_tensor("ind", [HPG, NT, P], BF16, kind="ExternalInput").ap()
    vones = nc.dram_tensor("vones", [P, TT, HPG], BF16,
                           kind="ExternalInput").ap()
    outT = nc.dram_tensor("outT", [C, nq], F32, kind="ExternalOutput").ap()
    aps = (qxT, kvxT, wq, wk, wv, wp, msk, colsel, bcast, ind, vones, outT)
    with _FixedTileContext(nc) as tc:
        _body(tc, aps, nq)
    return nc


def _plan_compaction(attn_mask, nq_min=768):
    """Per-batch unmasked-token indices and a shared padded capacity."""
    sels = [np.nonzero(np.asarray(attn_mask[b]))[0] for b in range(B)]
    cnts = [len(s) for s in sels]
    cap = max(max(cnts) + 2, nq_min)
    cap = ((cap + 255) // 256) * 256
    return sels, cnts, cap


def make_in_maps(q_x, kv_x, attn_mask, Wq, Wkv, Wp, sels, cnts, nq):
    colsel = np.zeros((P, NT, HPG), np.float32)
    bcast = np.zeros((HPG, NT, P), np.float32)
    for t in range(NT):
        for pp in range(P):
            colsel[pp, t, 2 * t + pp // HD] = 1.0 / HD
            bcast[2 * t + pp // HD, t, pp] = 1.0
    ind = np.zeros((HPG, NT, P), np.float32)
    for t in range(NT):
        for pp in range(P):
            ind[2 * t + pp // HD, t, pp] = 1.0
    bf = lambda a: np.ascontiguousarray(a).astype(NPBF16)

    in_maps = []
    for core in range(NCORES):
        b, g = core // G, core % G
        sl = slice(g * CL, (g + 1) * CL)
        qc = np.zeros((C, nq), np.float32)
        qc[:, 0:cnts[b]] = q_x[b][sels[b]].T
        mv = np.zeros((nq,), np.float32)
        mv[0:cnts[b]] = SCALE
        in_maps.append({
            "qxT": bf(qc),
            "kvxT": bf(kv_x[b].T),
            "wq": bf(Wq[sl].T),
            "wk": bf(Wkv[sl].T),
            "wv": bf(Wkv[C + g * CL:C + (g + 1) * CL].T),
            "wp": bf(Wp[:, sl].T),
            "msk": np.broadcast_to(mv, (HPG, nq)).copy(),
            "colsel": bf(colsel),
            "bcast": bf(bcast),
            "ind": bf(ind),
            "vones": bf(np.ones((P, TT, HPG), np.float32)),
        })
    return in_maps


_NC_CACHE = {}


def get_nc(nq):
    if nq not in _NC_CACHE:
        _NC_CACHE[nq] = build_bass(nq)
    return _NC_CACHE[nq]


def kernel(q_x, kv_x, attn_mask, Wq, Wkv, qn_w, qn_b, kn_w, kn_b, Wp, bp,
           _profile=None):
    q_x = np.asarray(q_x, np.float32)
    kv_x = np.asarray(kv_x, np.float32)
    attn_mask = np.asarray(attn_mask)
    Wq = np.asarray(Wq, np.float32)
    Wkv = np.asarray(Wkv, np.float32)
    Wp = np.asarray(Wp, np.float32)
    bp = np.asarray(bp, np.float32)
    if not (np.all(np.asarray(qn_w) == 1) and np.all(np.asarray(qn_b) == 0)
            and np.all(np.asarray(kn_w) == 1) and np.all(np.asarray(kn_b) == 0)):
        raise NotImplementedError("kernel specialized to identity q/k norms")

    sels, cnts, nq = _plan_compaction(attn_mask)
    nc = get_nc(nq)
    in_maps = make_in_maps(q_x, kv_x, attn_mask, Wq, Wkv, Wp, sels, cnts, nq)
    res = bass_utils.run_bass_kernel_spmd(
        nc, in_maps, core_ids=list(range(NCORES)))
    if _profile is not None:
        _profile.append(res)
    out = np.empty((B, N, C), np.float32)
    for b in range(B):
        acc = res.results[G * b]["outT"] + res.results[G * b + 1]["outT"]
        out[b] = acc[:, cnts[b]][None, :] + bp   # uniform row for masked
        out[b, sels[b]] = acc[:, 0:cnts[b]].T + bp
    return out


# revision 21
# speedup vs baseline: 1.1370x; 1.1370x over previous
"""Cross-attention (B=4, N=2048, C=768, H=12, HD=64) on 8 TRN2 NeuronCores.

Sharding: core = (batch, head_group) with 4 batches x 2 groups of 6 heads.
Each core computes its group's Q/K/V projections, per-head-dim LayerNorm,
attention, and a partial output projection; the host sums the two group
partials per batch and adds the bias.

Key optimizations over the f32r baseline:
 - All matmul operands are bf16 (PSUM accumulation stays f32).  The PE
   processes one moving row per cycle either way, but bf16 halves SBUF
   footprint, weight-load time, and DMA traffic.
 - Query-token compaction: the reference masks ~50% of QUERY rows, and a
   masked row's output is exactly the uniform average of v (its score row
   is all zeros after the mask is folded into the LN scale).  The host
   gathers unmasked q tokens, the kernel runs attention on only NQ ~ 1280
   columns, and any PAD column (rs=0) computes precisely the uniform-
   attention output -- so the host reads column cnt_b as the shared
   output for all masked tokens of batch b, then scatters.
 - Attention scores are computed transposed (S^T[k_tok, q_tok]); softmax
   exp needs no row-max (LN bounds |S|); the denominator comes free from
   a ones-column appended to v.
"""

import numpy as np
import ml_dtypes

import concourse.bass as bass
import concourse.mybir as mybir
from concourse import tile
from concourse import bass_utils
from concourse.tile_scheduler import N_PROCS
from concourse.vector_clock import ScopedClock, VectorClock

F32 = mybir.dt.float32
BF16 = mybir.dt.bfloat16
AF = mybir.ActivationFunctionType
OP = mybir.AluOpType
NPBF16 = ml_dtypes.bfloat16

B, N, C, H, HD = 4, 2048, 768, 12, 64
G = 2                 # head groups (tensor parallel)
HPG = H // G          # 6 heads per group
CL = HPG * HD         # 384 local channels
P = 128
NT = CL // P          # 3 output tiles per group
CT = C // P           # 6 contraction tiles
TT = N // P           # 16 k-token tiles
EPS = 1e-5
SCALE = HD ** -0.5
NCORES = 8

_nop_ctr = [0]


class _FixedTileContext(tile.TileContext):
    """Workaround for a walrus build that allows at most ONE sync-wait per
    instruction: split multi-wait instructions into single-wait NoOps on the
    same engine, and emit the kernel-tail drain's waits as a nop chain."""

    def _split_multiwait(self, insts):
        out = []
        for inst in insts:
            si = getattr(inst, "sync_info", None)
            waits = list(si.on_wait) if si is not None and si.on_wait else []
            if len(waits) > 1:
                eng = inst.engine
                for w in waits[:-1]:
                    _nop_ctr[0] += 1
                    nop = mybir.InstNoOp(
                        name=f"I-waitsplit-{_nop_ctr[0]}", ins=[], outs=[]
                    )
                    nop.engine = eng
                    nop.sync_info = mybir.SyncInfo(on_wait=[w], on_update=[])
                    self.nc.register_instruction(nop)
                    out.append(nop)
                inst.sync_info = mybir.SyncInfo(
                    on_wait=[waits[-1]], on_update=list(si.on_update)
                )
            out.append(inst)
        return out

    def _lower_ordered_insts(self, ordered):
        ordered = {bb: self._split_multiwait(ins) for bb, ins in ordered.items()}
        super()._lower_ordered_insts(ordered)

    def _drain_and_barrier(self, tick_clock, wait_clock):
        gc = tick_clock.global_clock
        vals = [gc[p] for p in range(N_PROCS)]
        for p in [q for q, v in enumerate(vals) if v > 0]:
            partial = VectorClock(
                [vals[q] if q == p else 0 for q in range(N_PROCS)]
            )
            nop = self.nc.sync.nop(nofuse=True, hint="tail_drain_wait")
            wait_clock.add_sem_waits(nop.ins, ScopedClock({None: partial}))
        self.nc.sync.drain()
        self.nc.all_engine_barrier()
        assert self.sems is not None
        popped = self.nc._tile_sem_poison_stack.pop()
        assert popped is self._sem_poison
        self.nc.clear_and_free_semaphores(list(self.sems.allocated().values()))
        self.nc.all_engine_barrier()


def _mm(nc, out, lhsT, rhs, start, stop):
    nc.tensor.matmul(
        out, lhsT, rhs, start=start, stop=stop, skip_group_check=True
    )


def _chunks(total):
    """Split token range into chunks of 512 with a trailing 256 if needed."""
    out = []
    off = 0
    while off < total:
        w = 512 if total - off >= 512 else total - off
        out.append((off, w))
        off += w
    return out


def _body(tc, aps, nq):
    nc = tc.nc
    qxT, kvxT, wq, wk, wv, wp, msk, colsel, bcast, ind, vones, outT = aps

    cpool = tc.alloc_tile_pool(name="consts", bufs=1)
    bpool = tc.alloc_tile_pool(name="big", bufs=1)
    w_pool = tc.alloc_tile_pool(name="wts", bufs=1)

    # weights + first x tiles first in the DMA queue, consts after
    wq_sb = w_pool.tile([P, CT, CL], BF16, name="wq", tag="wq")
    nc.sync.dma_start(wq_sb[:], wq.rearrange("(ct p) m -> p ct m", p=P))
    wp_sb = bpool.tile([P, NT, C], BF16, name="wp", tag="wp")

    q_sb = [bpool.tile([P, nq], BF16, name=f"q{t}", tag=f"q{t}") for t in range(NT)]
    k_sb = [bpool.tile([P, N], BF16, name=f"k{t}", tag=f"k{t}") for t in range(NT)]
    v_sb = bpool.tile([P, TT, HPG, HD + 1], BF16, name="v", tag="v")
    den_all = bpool.tile([65, HPG * 512], F32, name="den", tag="den")

    q_chunks = _chunks(nq)
    k_chunks = _chunks(N)

    # ---------------- phase 1: projections + layernorm ----------------
    ps_t = tc.alloc_tile_pool(name="ps1", bufs=8, space="PSUM")
    xq_pool = tc.alloc_tile_pool(name="xq", bufs=3)
    xkv_pool = tc.alloc_tile_pool(name="xkv", bufs=7)
    sq_pool = tc.alloc_tile_pool(name="sq", bufs=3)
    st_pool = tc.alloc_tile_pool(name="st", bufs=2)
    rs_pool = tc.alloc_tile_pool(name="rs", bufs=3)

    def ln_chunk_a(xT, w_sb, dst, masked, co, cw):
        """Projection + LN stats; returns context for the apply step."""
        cs = slice(co, co + cw)
        pp = [ps_t.tile([P, cw], F32, name="pt", tag="pt") for _ in range(NT)]
        xts = []
        pool = xq_pool if masked else xkv_pool
        xtag = "xq" if masked else "xkv"
        for ct in range(CT):
            xt = pool.tile([P, cw], BF16, name=xtag, tag=xtag)
            nc.sync.dma_start(xt[:], xT[ct * P:(ct + 1) * P, cs])
            xts.append(xt)
            for t in range(NT):
                _mm(nc, pp[t][:], w_sb[:, ct, t * P:(t + 1) * P],
                    xt[:], ct == 0, ct == CT - 1)
        sqs = []
        for t in range(NT):
            nc.vector.tensor_copy(dst[t][:, cs], pp[t][:])
            sq_t = sq_pool.tile([P, cw], BF16, name="sq", tag="sq")
            nc.scalar.activation(sq_t[:], pp[t][:], AF.Square)
            sqs.append(sq_t)
        mu_ps = ps_t.tile([HPG, cw], F32, name="pt", tag="pt")
        for t in range(NT):
            _mm(nc, mu_ps[:], colsel_sb[:, t, :], dst[t][:, cs],
                t == 0, t == NT - 1)
        ms_ps = ps_t.tile([HPG, cw], F32, name="pt", tag="pt")
        for t in range(NT):
            _mm(nc, ms_ps[:], colsel_sb[:, t, :], sqs[t][:],
                t == 0, t == NT - 1)
        st = st_pool.tile([HPG, 2 * cw], F32, name="st", tag="st")
        work = st[:, 0:cw]
        mu_sb = st[:, cw:2 * cw]
        rsm = rs_pool.tile([HPG, 2, cw], BF16, name="rsm", tag="rsm")
        rs = rsm[:, 0, :]
        murs = rsm[:, 1, :]
        nc.vector.tensor_copy(mu_sb, mu_ps[:])
        # var = E[x^2] - mu^2
        nc.vector.scalar_tensor_tensor(
            work, mu_sb, 1.0, mu_sb, OP.mult, OP.mult)
        nc.vector.tensor_tensor(work, ms_ps[:], work, OP.subtract)
        # rs = (var + eps)^-0.5 = exp(-0.5 * ln(var + eps))
        nc.scalar.activation(work, work, AF.Ln, bias=eps_sb[:])
        if masked:
            nc.scalar.activation(work, work, AF.Exp, scale=-0.5)
            # fold attn scale + query mask into rs
            nc.vector.tensor_tensor(rs, work, msk_sb[:, cs], OP.mult)
        else:
            nc.scalar.activation(rs, work, AF.Exp, scale=-0.5)
        # murs = -mu * rs
        nc.vector.scalar_tensor_tensor(
            murs, mu_sb, -1.0, rs, OP.mult, OP.mult)
        if not masked:
            # v projection reuses this chunk's kv x-tiles
            for tl in range(cw // P):
                tt = co // P + tl
                vp = ps_t.tile([P, CL], F32, name="pt", tag="pt")
                for ct in range(CT):
                    _mm(nc, vp[:], xts[ct][:, tl * P:(tl + 1) * P],
                        wv_sb[:, ct, :], ct == 0, ct == CT - 1)
                nc.vector.tensor_copy(
                    v_sb[:, tt, :, 0:HD],
                    vp[:].rearrange("p (h d) -> p h d", h=HPG))
        return (dst, rs, murs, co, cw)

    def ln_chunk_b(ctx):
        """Broadcast rs/murs across head partitions and normalize dst."""
        dst, rs, murs, co, cw = ctx
        cs = slice(co, co + cw)
        for t in range(NT):
            rrep = ps_t.tile([P, cw], F32, name="pt", tag="pt")
            _mm(nc, rrep[:], bcast_sb[:, t, :], rs, True, True)
            mrep = ps_t.tile([P, cw], F32, name="pt", tag="pt")
            _mm(nc, mrep[:], bcast_sb[:, t, :], murs, True, True)
            nc.vector.tensor_tensor(
                dst[t][:, cs], dst[t][:, cs], rrep[:], OP.mult)
            nc.vector.tensor_tensor(
                dst[t][:, cs], dst[t][:, cs], mrep[:], OP.add)

    if True:
        # schedule: interleave q/k chunks, software-pipelined by one stage
        sched = []
        for c in range(len(k_chunks)):
            if c < len(q_chunks):
                sched.append((qxT, None, q_sb, True, q_chunks[c]))
            sched.append((kvxT, None, k_sb, False, k_chunks[c]))

        colsel_sb = cpool.tile([P, NT, HPG], BF16, name="colsel", tag="colsel")
        nc.sync.dma_start(colsel_sb[:], colsel[:])
        bcast_sb = cpool.tile([HPG, NT, P], BF16, name="bcast", tag="bcast")
        nc.sync.dma_start(bcast_sb[:], bcast[:])
        msk_sb = cpool.tile([HPG, nq], F32, name="msk", tag="msk")
        nc.sync.dma_start(msk_sb[:], msk[:])
        eps_sb = cpool.tile([HPG, 1], F32, name="eps", tag="eps")
        nc.vector.memset(eps_sb[:], EPS)
        wk_sb = w_pool.tile([P, CT, CL], BF16, name="wk", tag="wk")
        nc.sync.dma_start(wk_sb[:], wk.rearrange("(ct p) m -> p ct m", p=P))
        wv_sb = w_pool.tile([P, CT, CL], BF16, name="wv", tag="wv")
        nc.sync.dma_start(wv_sb[:], wv.rearrange("(ct p) m -> p ct m", p=P))

        prev = None
        for i, (xT, _, dst, masked, (co, cw)) in enumerate(sched):
            w_sb = wq_sb if masked else wk_sb
            cur = ln_chunk_a(xT, w_sb, dst, masked, co, cw)
            if i == 0:
                nc.sync.dma_start(v_sb[:, :, :, HD], vones[:])
            if prev is not None:
                ln_chunk_b(prev)
            prev = cur
        ln_chunk_b(prev)

    for pool in (rs_pool, st_pool, sq_pool, xkv_pool, xq_pool, w_pool, ps_t):
        pool.release()

    # ---------------- phase 2: attention + output projection ----------
    ps_s = tc.alloc_tile_pool(name="ps_s", bufs=2, space="PSUM")
    ps_o = tc.alloc_tile_pool(name="ps_o", bufs=2, space="PSUM")
    ps_t = tc.alloc_tile_pool(name="ps2", bufs=2, space="PSUM")
    e_pool = tc.alloc_tile_pool(name="e", bufs=4)
    o_pool = tc.alloc_tile_pool(name="o", bufs=2)
    den_pool = tc.alloc_tile_pool(name="dn", bufs=2)
    rcp_pool = tc.alloc_tile_pool(name="rcp", bufs=2)
    out_pool = tc.alloc_tile_pool(name="ot", bufs=3)
    if True:
        nc.sync.dma_start(wp_sb[:], wp.rearrange("(t p) m -> p t m", p=P))
        ind_sb = cpool.tile([HPG, NT, P], BF16, name="ind", tag="ind")
        nc.sync.dma_start(ind_sb[:], ind[:])

        def finish(ctx):
            """Normalize o_t by the softmax denominators and project out."""
            o_t, db, co, cw = ctx
            qs = slice(co, co + cw)
            den6 = rcp_pool.tile([HPG, cw], F32, name="den6", tag="den6")
            nc.sync.dma_start(den6[:], den_all[db:db + 1, 0:HPG * cw])
            rcp6 = rcp_pool.tile([HPG, cw], BF16, name="rcp6", tag="rcp6")
            with nc.allow_low_precision(reason="bf16 softmax denom recip"):
                nc.vector.reciprocal(rcp6[:], den6[:])
            for t in range(NT):
                rrep = ps_t.tile([P, cw], F32, name="pt", tag="pt")
                _mm(nc, rrep[:], ind_sb[:, t, :], rcp6[:], True, True)
                nc.vector.tensor_tensor(
                    o_t[t][:], o_t[t][:], rrep[:], OP.mult)
            for m in range(C // P):
                pp = ps_t.tile([P, cw], F32, name="pt", tag="pt")
                for t in range(NT):
                    _mm(nc, pp[:], wp_sb[:, t, m * P:(m + 1) * P],
                        o_t[t][:], t == 0, t == NT - 1)
                ot = out_pool.tile([P, cw], F32, name="ot", tag="ot")
                nc.vector.tensor_copy(ot[:], pp[:])
                nc.sync.dma_start(outT[m * P:(m + 1) * P, qs], ot[:])

        pending = None
        for qc, (co, cw) in enumerate(q_chunks):
            qs = slice(co, co + cw)
            kt_grp = 1024 // cw          # k-tiles per [128, 1024] exp group
            o_t = [o_pool.tile([P, cw], BF16, name=f"o{t}", tag=f"o{t}")
                   for t in range(NT)]
            db = 32 * (qc % 3)
            for hp in range(HPG // 2):
                t = hp
                po = [ps_o.tile([HD + 1, cw], F32, name="po", tag="po")
                      for _ in range(2)]
                for kg in range(TT // kt_grp):
                    sps, es = [], []
                    for hh in range(2):
                        off = hh * HD
                        sp = ps_s.tile([P, kt_grp * cw], F32,
                                       name="sp", tag="sp")
                        for j in range(kt_grp):
                            kt = kg * kt_grp + j
                            _mm(nc, sp[:, j * cw:(j + 1) * cw],
                                k_sb[t][off:off + HD, kt * P:(kt + 1) * P],
                                q_sb[t][off:off + HD, qs],
                                True, True)
                        sps.append(sp)
                    for hh in range(2):
                        e = e_pool.tile([P, kt_grp * cw], BF16,
                                        name="e", tag="e")
                        nc.scalar.activation(e[:], sps[hh][:], AF.Exp)
                        es.append(e)
                    for hh in range(2):
                        h = 2 * hp + hh
                        for j in range(kt_grp):
                            kt = kg * kt_grp + j
                            _mm(nc, po[hh][:], v_sb[:, kt, h, :],
                                es[hh][:, j * cw:(j + 1) * cw],
                                kt == 0, kt == TT - 1)
                for hh in range(2):
                    h = 2 * hp + hh
                    off = hh * HD
                    nc.vector.tensor_copy(
                        den_all[db:db + 1, h * cw:(h + 1) * cw],
                        po[hh][HD:HD + 1, :])
                    nc.vector.tensor_copy(
                        o_t[t][off:off + HD, :], po[hh][0:HD, :])
                if hp == 0 and pending is not None:
                    finish(pending)
                    pending = None
            pending = (o_t, db, co, cw)
        finish(pending)

    for pool in (out_pool, rcp_pool, den_pool, o_pool, e_pool,
                 ps_t, ps_o, ps_s, bpool, cpool):
        pool.release()


def build_bass(nq):
    nc = bass.Bass(trn_type="TRN2", debug=False, num_devices=NCORES)
    qxT = nc.dram_tensor("qxT", [C, nq], BF16, kind="ExternalInput").ap()
    kvxT = nc.dram_tensor("kvxT", [C, N], BF16, kind="ExternalInput").ap()
    wq = nc.dram_tensor("wq", [C, CL], BF16, kind="ExternalInput").ap()
    wk = nc.dram_tensor("wk", [C, CL], BF16, kind="ExternalInput").ap()
    wv = nc.dram_tensor("wv", [C, CL], BF16, kind="ExternalInput").ap()
    wp = nc.dram_tensor("wp", [CL, C], BF16, kind="ExternalInput").ap()
    msk = nc.dram_tensor("msk", [HPG, nq], F32, kind="ExternalInput").ap()
    colsel = nc.dram_tensor("colsel", [P, NT, HPG], BF16,
                            kind="ExternalInput").ap()
    bcast = nc.dram_tensor("bcast", [HPG, NT, P], BF16,
                           kind="ExternalInput").ap()
    ind = nc.dram_tensor("ind", [HPG, NT, P], BF16, kind="ExternalInput").ap()
    vones = nc.dram_tensor("vones", [P, TT, HPG], BF16,
                           kind="ExternalInput").ap()
    outT = nc.dram_tensor("outT", [C, nq], F32, kind="ExternalOutput").ap()
    aps = (qxT, kvxT, wq, wk, wv, wp, msk, colsel, bcast, ind, vones, outT)
    with _FixedTileContext(nc) as tc:
        _body(tc, aps, nq)
    return nc


def _plan_compaction(attn_mask, nq_min=768):
    """Per-batch unmasked-token indices and a shared padded capacity."""
    sels = [np.nonzero(np.asarray(attn_mask[b]))[0] for b in range(B)]
    cnts = [len(s) for s in sels]
    cap = max(max(cnts) + 2, nq_min)
    cap = ((cap + 255) // 256) * 256
    return sels, cnts, cap


def make_in_maps(q_x, kv_x, attn_mask, Wq, Wkv, Wp, sels, cnts, nq):
    colsel = np.zeros((P, NT, HPG), np.float32)
    bcast = np.zeros((HPG, NT, P), np.float32)
    for t in range(NT):
        for pp in range(P):
            colsel[pp, t, 2 * t + pp // HD] = 1.0 / HD
            bcast[2 * t + pp // HD, t, pp] = 1.0
    ind = np.zeros((HPG, NT, P), np.float32)
    for t in range(NT):
        for pp in range(P):
            ind[2 * t + pp // HD, t, pp] = 1.0
    bf = lambda a: np.ascontiguousarray(a).astype(NPBF16)

    in_maps = []
    for core in range(NCORES):
        b, g = core // G, core % G
        sl = slice(g * CL, (g + 1) * CL)
        qc = np.zeros((C, nq), np.float32)
        qc[:, 0:cnts[b]] = q_x[b][sels[b]].T
        mv = np.zeros((nq,), np.float32)
        mv[0:cnts[b]] = SCALE
        in_maps.append({
            "qxT": bf(qc),
            "kvxT": bf(kv_x[b].T),
            "wq": bf(Wq[sl].T),
            "wk": bf(Wkv[sl].T),
            "wv": bf(Wkv[C + g * CL:C + (g + 1) * CL].T),
            "wp": bf(Wp[:, sl].T),
            "msk": np.broadcast_to(mv, (HPG, nq)).copy(),
            "colsel": bf(colsel),
            "bcast": bf(bcast),
            "ind": bf(ind),
            "vones": bf(np.ones((P, TT, HPG), np.float32)),
        })
    return in_maps


_NC_CACHE = {}


def get_nc(nq):
    if nq not in _NC_CACHE:
        _NC_CACHE[nq] = build_bass(nq)
    return _NC_CACHE[nq]


def kernel(q_x, kv_x, attn_mask, Wq, Wkv, qn_w, qn_b, kn_w, kn_b, Wp, bp,
           _profile=None):
    q_x = np.asarray(q_x, np.float32)
    kv_x = np.asarray(kv_x, np.float32)
    attn_mask = np.asarray(attn_mask)
    Wq = np.asarray(Wq, np.float32)
    Wkv = np.asarray(Wkv, np.float32)
    Wp = np.asarray(Wp, np.float32)
    bp = np.asarray(bp, np.float32)
    if not (np.all(np.asarray(qn_w) == 1) and np.all(np.asarray(qn_b) == 0)
            and np.all(np.asarray(kn_w) == 1) and np.all(np.asarray(kn_b) == 0)):
        raise NotImplementedError("kernel specialized to identity q/k norms")

    sels, cnts, nq = _plan_compaction(attn_mask)
    nc = get_nc(nq)
    in_maps = make_in_maps(q_x, kv_x, attn_mask, Wq, Wkv, Wp, sels, cnts, nq)
    res = bass_utils.run_bass_kernel_spmd(
        nc, in_maps, core_ids=list(range(NCORES)))
    if _profile is not None:
        _profile.append(res)
    out = np.empty((B, N, C), np.float32)
    for b in range(B):
        acc = res.results[G * b]["outT"] + res.results[G * b + 1]["outT"]
        out[b] = acc[:, cnts[b]][None, :] + bp   # uniform row for masked
        out[b, sels[b]] = acc[:, 0:cnts[b]].T + bp
    return out


# revision 22
# speedup vs baseline: 1.1692x; 1.0283x over previous
"""Cross-attention (B=4, N=2048, C=768, H=12, HD=64) on 8 TRN2 NeuronCores.

Sharding: core = (batch, head_group) with 4 batches x 2 groups of 6 heads.
Each core computes its group's Q/K/V projections, per-head-dim LayerNorm,
attention, and a partial output projection; the host sums the two group
partials per batch and adds the bias.

Key optimizations over the f32r baseline:
 - All matmul operands are bf16 (PSUM accumulation stays f32).  The PE
   processes one moving row per cycle either way, but bf16 halves SBUF
   footprint, weight-load time, and DMA traffic.
 - Query-token compaction: the reference masks ~50% of QUERY rows, and a
   masked row's output is exactly the uniform average of v (its score row
   is all zeros after the mask is folded into the LN scale).  The host
   gathers unmasked q tokens, the kernel runs attention on only NQ ~ 1280
   columns, and any PAD column (rs=0) computes precisely the uniform-
   attention output -- so the host reads column cnt_b as the shared
   output for all masked tokens of batch b, then scatters.
 - Attention scores are computed transposed (S^T[k_tok, q_tok]); softmax
   exp needs no row-max (LN bounds |S|); the denominator comes free from
   a ones-column appended to v.
"""

import numpy as np
import ml_dtypes

import concourse.bass as bass
import concourse.mybir as mybir
from concourse import tile
from concourse import bass_utils
from concourse.tile_scheduler import N_PROCS
from concourse.vector_clock import ScopedClock, VectorClock

F32 = mybir.dt.float32
BF16 = mybir.dt.bfloat16
AF = mybir.ActivationFunctionType
OP = mybir.AluOpType
NPBF16 = ml_dtypes.bfloat16

B, N, C, H, HD = 4, 2048, 768, 12, 64
G = 2                 # head groups (tensor parallel)
HPG = H // G          # 6 heads per group
CL = HPG * HD         # 384 local channels
P = 128
NT = CL // P          # 3 output tiles per group
CT = C // P           # 6 contraction tiles
TT = N // P           # 16 k-token tiles
EPS = 1e-5
SCALE = HD ** -0.5
NCORES = 8

_nop_ctr = [0]


class _FixedTileContext(tile.TileContext):
    """Workaround for a walrus build that allows at most ONE sync-wait per
    instruction: split multi-wait instructions into single-wait NoOps on the
    same engine, and emit the kernel-tail drain's waits as a nop chain."""

    def _split_multiwait(self, insts):
        out = []
        for inst in insts:
            si = getattr(inst, "sync_info", None)
            waits = list(si.on_wait) if si is not None and si.on_wait else []
            if len(waits) > 1:
                eng = inst.engine
                for w in waits[:-1]:
                    _nop_ctr[0] += 1
                    nop = mybir.InstNoOp(
                        name=f"I-waitsplit-{_nop_ctr[0]}", ins=[], outs=[]
                    )
                    nop.engine = eng
                    nop.sync_info = mybir.SyncInfo(on_wait=[w], on_update=[])
                    self.nc.register_instruction(nop)
                    out.append(nop)
                inst.sync_info = mybir.SyncInfo(
                    on_wait=[waits[-1]], on_update=list(si.on_update)
                )
            out.append(inst)
        return out

    def _lower_ordered_insts(self, ordered):
        ordered = {bb: self._split_multiwait(ins) for bb, ins in ordered.items()}
        super()._lower_ordered_insts(ordered)

    def _drain_and_barrier(self, tick_clock, wait_clock):
        gc = tick_clock.global_clock
        vals = [gc[p] for p in range(N_PROCS)]
        for p in [q for q, v in enumerate(vals) if v > 0]:
            partial = VectorClock(
                [vals[q] if q == p else 0 for q in range(N_PROCS)]
            )
            nop = self.nc.sync.nop(nofuse=True, hint="tail_drain_wait")
            wait_clock.add_sem_waits(nop.ins, ScopedClock({None: partial}))
        self.nc.sync.drain()
        self.nc.all_engine_barrier()
        assert self.sems is not None
        popped = self.nc._tile_sem_poison_stack.pop()
        assert popped is self._sem_poison
        self.nc.clear_and_free_semaphores(list(self.sems.allocated().values()))
        self.nc.all_engine_barrier()


def _mm(nc, out, lhsT, rhs, start, stop):
    nc.tensor.matmul(
        out, lhsT, rhs, start=start, stop=stop, skip_group_check=True
    )


def _chunks(total):
    """Split token range into chunks of 512 with a trailing 256 if needed."""
    out = []
    off = 0
    while off < total:
        w = 512 if total - off >= 512 else total - off
        out.append((off, w))
        off += w
    return out


def _body(tc, aps, nq):
    nc = tc.nc
    qxT, kvxT, wq, wk, wv, wp, msk, colsel, bcast, ind, vones, outT = aps

    cpool = tc.alloc_tile_pool(name="consts", bufs=1)
    bpool = tc.alloc_tile_pool(name="big", bufs=1)
    w_pool = tc.alloc_tile_pool(name="wts", bufs=1)

    # weights + first x tiles first in the DMA queue, consts after
    wq_sb = w_pool.tile([P, CT, CL], BF16, name="wq", tag="wq")
    nc.sync.dma_start(wq_sb[:], wq.rearrange("(ct p) m -> p ct m", p=P))
    wp_sb = bpool.tile([P, NT, C], BF16, name="wp", tag="wp")

    q_sb = [bpool.tile([P, nq], BF16, name=f"q{t}", tag=f"q{t}") for t in range(NT)]
    k_sb = [bpool.tile([P, N], BF16, name=f"k{t}", tag=f"k{t}") for t in range(NT)]
    v_sb = bpool.tile([P, TT, HPG, HD + 1], BF16, name="v", tag="v")
    den_all = bpool.tile([65, HPG * 512], F32, name="den", tag="den")

    q_chunks = _chunks(nq)
    k_chunks = _chunks(N)

    # ---------------- phase 1: projections + layernorm ----------------
    ps_t = tc.alloc_tile_pool(name="ps1", bufs=8, space="PSUM")
    xq_pool = tc.alloc_tile_pool(name="xq", bufs=3)
    xkv_pool = tc.alloc_tile_pool(name="xkv", bufs=7)
    sq_pool = tc.alloc_tile_pool(name="sq", bufs=3)
    st_pool = tc.alloc_tile_pool(name="st", bufs=2)
    rs_pool = tc.alloc_tile_pool(name="rs", bufs=3)

    def ln_chunk_a(xT, w_sb, dst, masked, co, cw):
        """Projection + LN stats; returns context for the apply step."""
        cs = slice(co, co + cw)
        pp = [ps_t.tile([P, cw], F32, name="pt", tag="pt") for _ in range(NT)]
        xts = []
        pool = xq_pool if masked else xkv_pool
        xtag = "xq" if masked else "xkv"
        for ct in range(CT):
            xt = pool.tile([P, cw], BF16, name=xtag, tag=xtag)
            nc.sync.dma_start(xt[:], xT[ct * P:(ct + 1) * P, cs])
            xts.append(xt)
            for t in range(NT):
                _mm(nc, pp[t][:], w_sb[:, ct, t * P:(t + 1) * P],
                    xt[:], ct == 0, ct == CT - 1)
        sqs = []
        for t in range(NT):
            nc.vector.tensor_copy(dst[t][:, cs], pp[t][:])
            sq_t = sq_pool.tile([P, cw], BF16, name="sq", tag="sq")
            nc.scalar.activation(sq_t[:], pp[t][:], AF.Square)
            sqs.append(sq_t)
        mu_ps = ps_t.tile([HPG, cw], F32, name="pt", tag="pt")
        for t in range(NT):
            _mm(nc, mu_ps[:], colsel_sb[:, t, :], dst[t][:, cs],
                t == 0, t == NT - 1)
        ms_ps = ps_t.tile([HPG, cw], F32, name="pt", tag="pt")
        for t in range(NT):
            _mm(nc, ms_ps[:], colsel_sb[:, t, :], sqs[t][:],
                t == 0, t == NT - 1)
        st = st_pool.tile([HPG, 2 * cw], F32, name="st", tag="st")
        work = st[:, 0:cw]
        mu_sb = st[:, cw:2 * cw]
        rsm = rs_pool.tile([HPG, 2, cw], BF16, name="rsm", tag="rsm")
        rs = rsm[:, 0, :]
        murs = rsm[:, 1, :]
        nc.vector.tensor_copy(mu_sb, mu_ps[:])
        # var = E[x^2] - mu^2
        nc.vector.scalar_tensor_tensor(
            work, mu_sb, 1.0, mu_sb, OP.mult, OP.mult)
        nc.vector.tensor_tensor(work, ms_ps[:], work, OP.subtract)
        # rs = (var + eps)^-0.5 = exp(-0.5 * ln(var + eps))
        nc.scalar.activation(work, work, AF.Ln, bias=eps_sb[:])
        if masked:
            nc.scalar.activation(work, work, AF.Exp, scale=-0.5)
            # fold attn scale + query mask into rs
            nc.vector.tensor_tensor(rs, work, msk_sb[:, cs], OP.mult)
        else:
            nc.scalar.activation(rs, work, AF.Exp, scale=-0.5)
        # murs = -mu * rs
        nc.vector.scalar_tensor_tensor(
            murs, mu_sb, -1.0, rs, OP.mult, OP.mult)
        if not masked:
            # v projection reuses this chunk's kv x-tiles
            for tl in range(cw // P):
                tt = co // P + tl
                vp = ps_t.tile([P, CL], F32, name="pt", tag="pt")
                for ct in range(CT):
                    _mm(nc, vp[:], xts[ct][:, tl * P:(tl + 1) * P],
                        wv_sb[:, ct, :], ct == 0, ct == CT - 1)
                nc.vector.tensor_copy(
                    v_sb[:, tt, :, 0:HD],
                    vp[:].rearrange("p (h d) -> p h d", h=HPG))
        return (dst, rs, murs, co, cw)

    def ln_chunk_b(ctx):
        """Broadcast rs/murs across head partitions and normalize dst."""
        dst, rs, murs, co, cw = ctx
        cs = slice(co, co + cw)
        for t in range(NT):
            rrep = ps_t.tile([P, cw], F32, name="pt", tag="pt")
            _mm(nc, rrep[:], bcast_sb[:, t, :], rs, True, True)
            mrep = ps_t.tile([P, cw], F32, name="pt", tag="pt")
            _mm(nc, mrep[:], bcast_sb[:, t, :], murs, True, True)
            nc.vector.tensor_tensor(
                dst[t][:, cs], dst[t][:, cs], rrep[:], OP.mult)
            nc.vector.tensor_tensor(
                dst[t][:, cs], dst[t][:, cs], mrep[:], OP.add)

    if True:
        # schedule: interleave q/k chunks, software-pipelined by one stage
        sched = []
        for c in range(len(k_chunks)):
            if c < len(q_chunks):
                sched.append((qxT, None, q_sb, True, q_chunks[c]))
            sched.append((kvxT, None, k_sb, False, k_chunks[c]))

        colsel_sb = cpool.tile([P, NT, HPG], BF16, name="colsel", tag="colsel")
        nc.sync.dma_start(colsel_sb[:], colsel[:])
        bcast_sb = cpool.tile([HPG, NT, P], BF16, name="bcast", tag="bcast")
        nc.sync.dma_start(bcast_sb[:], bcast[:])
        msk_sb = cpool.tile([HPG, nq], F32, name="msk", tag="msk")
        nc.sync.dma_start(msk_sb[:], msk[:])
        eps_sb = cpool.tile([HPG, 1], F32, name="eps", tag="eps")
        nc.vector.memset(eps_sb[:], EPS)
        # wk/wv ride the GpSimd DGE so they don't serialize behind the
        # x-tile stream on the SP queue
        wk_sb = w_pool.tile([P, CT, CL], BF16, name="wk", tag="wk")
        nc.gpsimd.dma_start(wk_sb[:], wk.rearrange("(ct p) m -> p ct m", p=P))
        wv_sb = w_pool.tile([P, CT, CL], BF16, name="wv", tag="wv")
        nc.gpsimd.dma_start(wv_sb[:], wv.rearrange("(ct p) m -> p ct m", p=P))

        prev = None
        for i, (xT, _, dst, masked, (co, cw)) in enumerate(sched):
            w_sb = wq_sb if masked else wk_sb
            cur = ln_chunk_a(xT, w_sb, dst, masked, co, cw)
            if i == 0:
                nc.sync.dma_start(v_sb[:, :, :, HD], vones[:])
            if prev is not None:
                ln_chunk_b(prev)
            prev = cur
        ln_chunk_b(prev)

    for pool in (rs_pool, st_pool, sq_pool, xkv_pool, xq_pool, w_pool, ps_t):
        pool.release()

    # ---------------- phase 2: attention + output projection ----------
    ps_s = tc.alloc_tile_pool(name="ps_s", bufs=2, space="PSUM")
    ps_o = tc.alloc_tile_pool(name="ps_o", bufs=2, space="PSUM")
    ps_t = tc.alloc_tile_pool(name="ps2", bufs=2, space="PSUM")
    e_pool = tc.alloc_tile_pool(name="e", bufs=4)
    o_pool = tc.alloc_tile_pool(name="o", bufs=2)
    den_pool = tc.alloc_tile_pool(name="dn", bufs=2)
    rcp_pool = tc.alloc_tile_pool(name="rcp", bufs=2)
    out_pool = tc.alloc_tile_pool(name="ot", bufs=3)
    if True:
        nc.sync.dma_start(wp_sb[:], wp.rearrange("(t p) m -> p t m", p=P))
        ind_sb = cpool.tile([HPG, NT, P], BF16, name="ind", tag="ind")
        nc.sync.dma_start(ind_sb[:], ind[:])

        def finish(ctx):
            """Normalize o_t by the softmax denominators and project out."""
            o_t, db, co, cw = ctx
            qs = slice(co, co + cw)
            den6 = rcp_pool.tile([HPG, cw], F32, name="den6", tag="den6")
            nc.sync.dma_start(den6[:], den_all[db:db + 1, 0:HPG * cw])
            rcp6 = rcp_pool.tile([HPG, cw], BF16, name="rcp6", tag="rcp6")
            with nc.allow_low_precision(reason="bf16 softmax denom recip"):
                nc.vector.reciprocal(rcp6[:], den6[:])
            for t in range(NT):
                rrep = ps_t.tile([P, cw], F32, name="pt", tag="pt")
                _mm(nc, rrep[:], ind_sb[:, t, :], rcp6[:], True, True)
                nc.vector.tensor_tensor(
                    o_t[t][:], o_t[t][:], rrep[:], OP.mult)
            for m in range(C // P):
                pp = ps_t.tile([P, cw], F32, name="pt", tag="pt")
                for t in range(NT):
                    _mm(nc, pp[:], wp_sb[:, t, m * P:(m + 1) * P],
                        o_t[t][:], t == 0, t == NT - 1)
                ot = out_pool.tile([P, cw], F32, name="ot", tag="ot")
                nc.vector.tensor_copy(ot[:], pp[:])
                nc.sync.dma_start(outT[m * P:(m + 1) * P, qs], ot[:])

        pending = None
        for qc, (co, cw) in enumerate(q_chunks):
            qs = slice(co, co + cw)
            kt_grp = 1024 // cw          # k-tiles per [128, 1024] exp group
            o_t = [o_pool.tile([P, cw], BF16, name=f"o{t}", tag=f"o{t}")
                   for t in range(NT)]
            db = 32 * (qc % 3)
            for hp in range(HPG // 2):
                t = hp
                po = [ps_o.tile([HD + 1, cw], F32, name="po", tag="po")
                      for _ in range(2)]
                for kg in range(TT // kt_grp):
                    sps, es = [], []
                    for hh in range(2):
                        off = hh * HD
                        sp = ps_s.tile([P, kt_grp * cw], F32,
                                       name="sp", tag="sp")
                        for j in range(kt_grp):
                            kt = kg * kt_grp + j
                            _mm(nc, sp[:, j * cw:(j + 1) * cw],
                                k_sb[t][off:off + HD, kt * P:(kt + 1) * P],
                                q_sb[t][off:off + HD, qs],
                                True, True)
                        sps.append(sp)
                    for hh in range(2):
                        e = e_pool.tile([P, kt_grp * cw], BF16,
                                        name="e", tag="e")
                        nc.scalar.activation(e[:], sps[hh][:], AF.Exp)
                        es.append(e)
                    for hh in range(2):
                        h = 2 * hp + hh
                        for j in range(kt_grp):
                            kt = kg * kt_grp + j
                            _mm(nc, po[hh][:], v_sb[:, kt, h, :],
                                es[hh][:, j * cw:(j + 1) * cw],
                                kt == 0, kt == TT - 1)
                for hh in range(2):
                    h = 2 * hp + hh
                    off = hh * HD
                    nc.vector.tensor_copy(
                        den_all[db:db + 1, h * cw:(h + 1) * cw],
                        po[hh][HD:HD + 1, :])
                    nc.vector.tensor_copy(
                        o_t[t][off:off + HD, :], po[hh][0:HD, :])
                if hp == 0 and pending is not None:
                    finish(pending)
                    pending = None
            pending = (o_t, db, co, cw)
        finish(pending)

    for pool in (out_pool, rcp_pool, den_pool, o_pool, e_pool,
                 ps_t, ps_o, ps_s, bpool, cpool):
        pool.release()


def build_bass(nq):
    nc = bass.Bass(trn_type="TRN2", debug=False, num_devices=NCORES)
    qxT = nc.dram_tensor("qxT", [C, nq], BF16, kind="ExternalInput").ap()
    kvxT = nc.dram_tensor("kvxT", [C, N], BF16, kind="ExternalInput").ap()
    wq = nc.dram_tensor("wq", [C, CL], BF16, kind="ExternalInput").ap()
    wk = nc.dram_tensor("wk", [C, CL], BF16, kind="ExternalInput").ap()
    wv = nc.dram_tensor("wv", [C, CL], BF16, kind="ExternalInput").ap()
    wp = nc.dram_tensor("wp", [CL, C], BF16, kind="ExternalInput").ap()
    msk = nc.dram_tensor("msk", [HPG, nq], F32, kind="ExternalInput").ap()
    colsel = nc.dram_tensor("colsel", [P, NT, HPG], BF16,
                            kind="ExternalInput").ap()
    bcast = nc.dram_tensor("bcast", [HPG, NT, P], BF16,
                           kind="ExternalInput").ap()
    ind = nc.dram_tensor("ind", [HPG, NT, P], BF16, kind="ExternalInput").ap()
    vones = nc.dram_tensor("vones", [P, TT, HPG], BF16,
                           kind="ExternalInput").ap()
    outT = nc.dram_tensor("outT", [C, nq], F32, kind="ExternalOutput").ap()
    aps = (qxT, kvxT, wq, wk, wv, wp, msk, colsel, bcast, ind, vones, outT)
    with _FixedTileContext(nc) as tc:
        _body(tc, aps, nq)
    return nc


def _plan_compaction(attn_mask, nq_min=768):
    """Per-batch unmasked-token indices and a shared padded capacity."""
    sels = [np.nonzero(np.asarray(attn_mask[b]))[0] for b in range(B)]
    cnts = [len(s) for s in sels]
    cap = max(max(cnts) + 2, nq_min)
    cap = ((cap + 255) // 256) * 256
    return sels, cnts, cap


def make_in_maps(q_x, kv_x, attn_mask, Wq, Wkv, Wp, sels, cnts, nq):
    colsel = np.zeros((P, NT, HPG), np.float32)
    bcast = np.zeros((HPG, NT, P), np.float32)
    for t in range(NT):
        for pp in range(P):
            colsel[pp, t, 2 * t + pp // HD] = 1.0 / HD
            bcast[2 * t + pp // HD, t, pp] = 1.0
    ind = np.zeros((HPG, NT, P), np.float32)
    for t in range(NT):
        for pp in range(P):
            ind[2 * t + pp // HD, t, pp] = 1.0
    bf = lambda a: np.ascontiguousarray(a).astype(NPBF16)

    in_maps = []
    for core in range(NCORES):
        b, g = core // G, core % G
        sl = slice(g * CL, (g + 1) * CL)
        qc = np.zeros((C, nq), np.float32)
        qc[:, 0:cnts[b]] = q_x[b][sels[b]].T
        mv = np.zeros((nq,), np.float32)
        mv[0:cnts[b]] = SCALE
        in_maps.append({
            "qxT": bf(qc),
            "kvxT": bf(kv_x[b].T),
            "wq": bf(Wq[sl].T),
            "wk": bf(Wkv[sl].T),
            "wv": bf(Wkv[C + g * CL:C + (g + 1) * CL].T),
            "wp": bf(Wp[:, sl].T),
            "msk": np.broadcast_to(mv, (HPG, nq)).copy(),
            "colsel": bf(colsel),
            "bcast": bf(bcast),
            "ind": bf(ind),
            "vones": bf(np.ones((P, TT, HPG), np.float32)),
        })
    return in_maps


_NC_CACHE = {}


def get_nc(nq):
    if nq not in _NC_CACHE:
        _NC_CACHE[nq] = build_bass(nq)
    return _NC_CACHE[nq]


def kernel(q_x, kv_x, attn_mask, Wq, Wkv, qn_w, qn_b, kn_w, kn_b, Wp, bp,
           _profile=None):
    q_x = np.asarray(q_x, np.float32)
    kv_x = np.asarray(kv_x, np.float32)
    attn_mask = np.asarray(attn_mask)
    Wq = np.asarray(Wq, np.float32)
    Wkv = np.asarray(Wkv, np.float32)
    Wp = np.asarray(Wp, np.float32)
    bp = np.asarray(bp, np.float32)
    if not (np.all(np.asarray(qn_w) == 1) and np.all(np.asarray(qn_b) == 0)
            and np.all(np.asarray(kn_w) == 1) and np.all(np.asarray(kn_b) == 0)):
        raise NotImplementedError("kernel specialized to identity q/k norms")

    sels, cnts, nq = _plan_compaction(attn_mask)
    nc = get_nc(nq)
    in_maps = make_in_maps(q_x, kv_x, attn_mask, Wq, Wkv, Wp, sels, cnts, nq)
    res = bass_utils.run_bass_kernel_spmd(
        nc, in_maps, core_ids=list(range(NCORES)))
    if _profile is not None:
        _profile.append(res)
    out = np.empty((B, N, C), np.float32)
    for b in range(B):
        acc = res.results[G * b]["outT"] + res.results[G * b + 1]["outT"]
        out[b] = acc[:, cnts[b]][None, :] + bp   # uniform row for masked
        out[b, sels[b]] = acc[:, 0:cnts[b]].T + bp
    return out
